# revision 78
# baseline (speedup 1.0000x reference)
"""Causal self-attention (GQA + rotary + qk-rmsnorm) on 8 TRN2 NeuronCores.

Sharding: tensor-parallel over (batch, kv-group).  Core c handles batch
b = c//4 and kv-group g = c%4 (4 q heads + 1 kv head), computing the
partial output  y_g @ Wo[256g:256(g+1), :].  The host sums the 4 group
partials per batch (the "all-reduce after c_proj" done at unshard time).

v3 layout (all-bf16, phase-pipelined; 176.7us vs 212.0us v2):
 - x, weights, cos/sin, and the output travel as bf16; f32 only in PSUM.
 - all inputs arrive as their SBUF images (host pre-transposed), so every
   input DMA is contiguous full-speed 2KB+ lines (v2's strided weight/x
   rearrange DMAs cost ~10us of serial startup).
 - projections run as chunk-PAIR blocks (1024 queries) accumulating into
   (128,1024) PSUM tiles borrowed from the lg pool: one DVE cast per pair
   and a dense warm-up matmul stream for the HAM clock gate.
 - k rotary uses a sign-swapped projection ([kA kB | kB -kA] columns of
   Wk) so the rotation is 3 full-width multiplies, no partition shifts;
   the kT rows 64-127 duplicate is a free SBUF->SBUF DMA.
 - the k-side rmsnorm scale and 1/sqrt(64) are folded into kT (PE row
   broadcast), so the attention exp has constant operands and both head
   halves share one (128,1024) PSUM tile -> single exp per key tile.
 - attention sweeps are Act(exp)-paced: pv trails qk by THREE key tiles
   (wt pool 6 deep) so the PE and Act queues never ping-pong; py is
   evacuated eagerly at sweep end and the normalize math (PE broadcast +
   Act ln/exp reciprocal + DVE muls) runs as a pumped filler, as does wo.
 - both proj pairs and ALL four stats blocks run before the sweeps (one
   contiguous warm-up matmul stream; the stats gpsimd sq-muls never queue
   ahead of the attention mask affines); per-chunk rotary preps slide in
   between sweeps where their DVE work hides under the exp stream.  The
   rotary processes both head pairs in full-width (128,512) DVE ops
   (free-dim-bound: same cost, half the instructions; tq tiles carry
   distinct pool tags so the late qpair reads never see slot reuse).
 - the causal mask affine_select covers only the 128-col diagonal block
   (base 0 there; everything past it is keys<=queries) -- 4x less gpsimd.
"""
import sys

sys.path.insert(0, "/opt/trn_rl_repo")

import math
import numpy as np
import ml_dtypes

# ---------------------------------------------------------------------------
# walrus compat layer (inlined): this environment's walrus build accepts at
# most ONE sync-wait command per instruction, and the Tile tail barrier's
# Drain(eq-wait + update) instructions don't encode.  Patch 1 hoists extra
# waits onto standalone InstEventSemaphore instructions; patch 2 replaces the
# tail with a ge-only drain + count-up barrier + semaphore clears.
# ---------------------------------------------------------------------------

import concourse.bass as bass
import concourse.mybir as mybir
import concourse.tile as tile_mod
from concourse.vector_clock import ScopedClock

_installed = False


def _mk_es(nc, engine, waits=(), updates=()):
    es = mybir.InstEventSemaphore(name=nc.get_next_instruction_name(), ins=[], outs=[])
    es.engine = engine
    es.sync_info = mybir.SyncInfo(on_wait=list(waits), on_update=list(updates))
    return es


def _legalize_lists(nc, ordered):
    n_hoisted = 0
    for bb_name, insts in ordered.items():
        new = []
        for inst in insts:
            si = inst.sync_info
            ws = list(si.on_wait) if si and si.on_wait else []
            if len(ws) > 1:
                for w in ws[:-1]:
                    es = _mk_es(nc, inst.engine, waits=[w])
                    nc.register_instruction(es, overwrite=True)
                    new.append(es)
                    n_hoisted += 1
                while len(si.on_wait) > 1:
                    si.on_wait.pop(0)
            new.append(inst)
        insts[:] = new
    return n_hoisted


def install():
    global _installed
    if _installed:
        return
    _installed = True

    orig_lower = tile_mod.TileContext._lower_ordered_insts

    def patched_lower(self, ordered):
        _legalize_lists(self.nc, ordered)
        return orig_lower(self, ordered)

    tile_mod.TileContext._lower_ordered_insts = patched_lower

    def patched_drain_and_barrier(self, tick_clock, wait_clock):
        nc = self.nc
        probe = mybir.InstEventSemaphore(name="wait-probe-unused", ins=[], outs=[])
        probe.engine = mybir.EngineType.SP
        wait_clock.add_sem_waits(probe, ScopedClock({None: tick_clock.global_clock}))
        ws = list(probe.sync_info.on_wait) if probe.sync_info and probe.sync_info.on_wait else []
        sp = nc.engines[mybir.EngineType.SP]
        for w in ws:
            sp.add_instruction(_mk_es(nc, mybir.EngineType.SP, waits=[w]))

        bar = nc.alloc_semaphore(f"tail_barrier_{nc.next_id()}")
        n_eng = 0
        for eng_type, eng in nc.engines.items():
            d = mybir.InstDrain(
                name=nc.get_next_instruction_name(), ins=[], outs=[],
                bass_is_fusable=False,
            )
            d.engine = eng_type
            eng.add_instruction(d)
            upd = mybir.SyncUpdate(
                sync_type="semaphore", id=bar.num,
                update_mode="sem-inc", update_value=1,
            )
            eng.add_instruction(_mk_es(nc, eng_type, updates=[upd]))
            n_eng += 1

        pool_eng = mybir.EngineType.Pool
        gw = mybir.SyncWait(
            sync_type="semaphore", id=bar.num,
            wait_mode="sem-ge-imm", wait_value=n_eng,
        )
        nc.engines[pool_eng].add_instruction(_mk_es(nc, pool_eng, waits=[gw]))

        popped = nc._tile_sem_poison_stack.pop()
        assert popped is self._sem_poison
        assert self.sems is not None
        tile_sems = list(self.sems.allocated().values())
        nc.clear_and_free_semaphores(tile_sems)
        nc.gpsimd.sem_clear(range(bar.num, bar.num + 1))

    tile_mod.TileContext._drain_and_barrier = patched_drain_and_barrier


install()

import concourse.tile as tile
from concourse.bass_utils import run_bass_kernel_spmd
from concourse.masks import make_identity

F32 = mybir.dt.float32
F32R = mybir.dt.float32r
BF16 = mybir.dt.bfloat16
F8 = mybir.dt.float8e4
AF = mybir.ActivationFunctionType
ALU = mybir.AluOpType
DR = mybir.MatmulPerfMode.DoubleRow

B, T, C = 2, 2048, 1024
N_HEAD, N_KV, HD = 16, 4, 64
HH = HD // 2  # 32, rotary half
NT = T // 512  # 4 chunks of 512
EPS = 1e-6
BLN8 = -math.log(8.0)
# softmax weights are stored fp8e4m3: exp(l - 4ln2) keeps the max (e^8)
# under the 448 fp8 ceiling; the uniform 1/16 factor cancels in y = num/den.
BLN16 = -4.0 * math.log(2.0)


def build_nc():
    nc = bass.Bass()
    # all inputs arrive pre-laid-out as their SBUF images (128 partitions x
    # contiguous free dim) so every load is a full-speed contiguous DMA
    xTs = nc.dram_tensor("xTs", (128, 16384), BF16, kind="ExternalInput")
    cosT4 = nc.dram_tensor("cosT4", (128, T), BF16, kind="ExternalInput")
    sinT4 = nc.dram_tensor("sinT4", (128, T), BF16, kind="ExternalInput")
    WqA = nc.dram_tensor("WqA", (128, 1024), BF16, kind="ExternalInput")
    WqB = nc.dram_tensor("WqB", (128, 1024), BF16, kind="ExternalInput")
    Wkv = nc.dram_tensor("Wkv", (128, 1024), BF16, kind="ExternalInput")
    Wv = nc.dram_tensor("Wv", (128, 512), BF16, kind="ExternalInput")
    Wo2 = nc.dram_tensor("Wo2", (256, C), BF16, kind="ExternalInput")
    outp = nc.dram_tensor("outp", (T, C), BF16, kind="ExternalOutput")

    with tile.TileContext(nc) as tc:
        with tc.tile_pool(name="persist", bufs=1) as pp, \
             tc.tile_pool(name="work", bufs=3) as wp, \
             tc.tile_pool(name="wtp", bufs=6) as wtp, \
             tc.tile_pool(name="osbp", bufs=4) as osbp, \
             tc.tile_pool(name="ps_mm", bufs=2, space="PSUM") as mmps, \
             tc.tile_pool(name="ps_lg", bufs=2, space="PSUM") as lgps, \
             tc.tile_pool(name="ps_py", bufs=2, space="PSUM") as pyps:
            # ---- constants ----
            identf = pp.tile([128, 128], F32)
            make_identity(nc, identf[:])
            identb = pp.tile([128, 128], BF16)
            nc.vector.tensor_copy(identb[:], identf[:])
            # E4 (128,4): E4[p,m] = 1 iff p//32 == m   (ms head-sum lhsT)
            e4f = pp.tile([128, 4], F32)
            nc.gpsimd.memset(e4f[:], 1.0)
            nc.gpsimd.affine_select(out=e4f[:], in_=e4f[:], compare_op=ALU.is_ge,
                                    fill=0.0, base=0, pattern=[[-32, 4]],
                                    channel_multiplier=1)
            nc.gpsimd.affine_select(out=e4f[:], in_=e4f[:], compare_op=ALU.is_ge,
                                    fill=0.0, base=31, pattern=[[32, 4]],
                                    channel_multiplier=-1)
            e4b = pp.tile([128, 4], BF16)
            nc.vector.tensor_copy(e4b[:], e4f[:])
            # SEL32 (4,128): SEL32[k,j] = 1 iff j//32 == k  (q-scale broadcast)
            sel32f = pp.tile([4, 128], F32)
            nc.gpsimd.memset(sel32f[:], 1.0)
            nc.gpsimd.affine_select(out=sel32f[:], in_=sel32f[:], compare_op=ALU.is_ge,
                                    fill=0.0, base=0, pattern=[[1, 128]],
                                    channel_multiplier=-32)
            nc.gpsimd.affine_select(out=sel32f[:], in_=sel32f[:], compare_op=ALU.is_ge,
                                    fill=0.0, base=31, pattern=[[-1, 128]],
                                    channel_multiplier=32)
            sel32b = pp.tile([4, 128], BF16)
            nc.vector.tensor_copy(sel32b[:], sel32f[:])
            # SEL64 (2,128): SEL64[k,j] = 1 iff j//64 == k  (denominator bcast)
            sel64f = pp.tile([2, 128], F32)
            nc.gpsimd.memset(sel64f[:], 1.0)
            nc.gpsimd.affine_select(out=sel64f[:], in_=sel64f[:], compare_op=ALU.is_ge,
                                    fill=0.0, base=0, pattern=[[1, 128]],
                                    channel_multiplier=-64)
            nc.gpsimd.affine_select(out=sel64f[:], in_=sel64f[:], compare_op=ALU.is_ge,
                                    fill=0.0, base=63, pattern=[[-1, 128]],
                                    channel_multiplier=64)
            sel64b = pp.tile([2, 128], BF16)
            nc.vector.tensor_copy(sel64b[:], sel64f[:])
            ones1_64 = pp.tile([1, 64], BF16)
            nc.gpsimd.memset(ones1_64[:], 1.0)
            ones64_1 = pp.tile([64, 1], BF16)
            nc.gpsimd.memset(ones64_1[:], 1.0)
            eps4 = pp.tile([4, 1], F32)
            nc.gpsimd.memset(eps4[:], EPS)
            eps1 = pp.tile([1, 1], F32)
            nc.gpsimd.memset(eps1[:], EPS)
            bln8_1 = pp.tile([1, 1], F32)
            nc.gpsimd.memset(bln8_1[:], BLN8)
            bln16_128 = pp.tile([128, 1], F32)
            nc.gpsimd.memset(bln16_128[:], BLN16)

            # ---- weights + rotary tables ----
            wAbig = pp.tile([128, 1024], BF16, name="wAbig")
            wBbig = pp.tile([128, 1024], BF16, name="wBbig")
            wKbig = pp.tile([128, 1024], BF16, name="wKbig")
            wVbig = pp.tile([128, 512], BF16, name="wVbig")
            # contiguous full-speed DMAs; only WqA ahead of the x tiles
            # (the first proj kind needs just wA + x, so x starts ~2us
            # earlier; wB/wK/wV still land before their kinds begin)
            nc.sync.dma_start(wAbig[:], WqA[:, :])
            wA_sb = [wAbig[:, 128 * k:128 * (k + 1)] for k in range(8)]
            wB_sb = [wBbig[:, 128 * k:128 * (k + 1)] for k in range(8)]
            wK_sb = [wKbig[:, 128 * k:128 * (k + 1)] for k in range(8)]
            wV_sb = [wVbig[:, 64 * k:64 * (k + 1)] for k in range(8)]
            Wo_sb = [pp.tile([128, C], BF16, name=f"wo{m}", tag=f"wo{m}") for m in range(2)]
            cos_sb = pp.tile([128, T], BF16)
            sin_sb = pp.tile([128, T], BF16)

            def emit_aux_dma():
                nc.sync.dma_start(cos_sb[:], cosT4[:])
                nc.sync.dma_start(sin_sb[:], sinT4[:])
                for m in range(2):
                    nc.sync.dma_start(Wo_sb[m][:], Wo2[128 * m:128 * (m + 1), :])

            # ---- persistent attention operands ----
            qpairc = [[pp.tile([128, 512], BF16, name=f"qpair{m}_{c}", tag=f"qpair{m}_{c}")
                       for c in range(NT)] for m in range(2)]
            kT2c = [pp.tile([128, 512], BF16, name=f"kT2_{c}", tag=f"kT2_{c}")
                    for c in range(NT)]
            v_aug = [pp.tile([128, HD + 1], BF16, name=f"vaug{i}", tag=f"vaug{i}")
                     for i in range(16)]
            for i in range(16):
                nc.gpsimd.memset(v_aug[i][:, 64:65], 1.0)
            yhat = [pp.tile([128, T], BF16, name=f"yhat{m}", tag=f"yhat{m}") for m in range(2)]
            # x row-tiles: xfull[j] (128, 2048) = xT rows 128j..128j+127 over
            # the whole sequence, so projections stream N=1024 query columns
            xfull = [pp.tile([128, 2048], BF16, name=f"xf{j}", tag=f"xf{j}")
                     for j in range(8)]

            def emit_xdma(h):
                # load the h-th half (1024 queries) of every row tile
                for j in range(8):
                    nc.sync.dma_start(
                        xfull[j][:, 1024 * h:1024 * h + 1024],
                        xTs[:, 2048 * j + 1024 * h:2048 * j + 1024 * h + 1024])

            # ---------------- filler machinery ----------------
            # Generators emit instructions lazily; the attention loop pumps
            # them between qk and pv so the PE never drains during exp waits.
            fillers = []  # list of generators, FIFO

            def pump(n):
                k = 0
                while k < n and fillers:
                    try:
                        next(fillers[0])
                        k += 1
                    except StopIteration:
                        fillers.pop(0)

            def drain_through(gen):
                # run generators from the head through `gen` to exhaustion
                while fillers:
                    g = fillers[0]
                    for _ in g:
                        pass
                    fillers.pop(0)
                    if g is gen:
                        break

            def drain_all():
                while fillers:
                    pump(1000000)

            def gen_proj2(h, outs):
                # project a 1024-query chunk pair (2h, 2h+1); N=1024 matmuls
                # halve the LDWEIGHTS + dispatch count.  PSUM comes from the
                # lg pool (2-bank slots; free outside the attention sweeps).
                for nm, wsb, rows in (("qA", wA_sb, 128), ("qB", wB_sb, 128),
                                      ("kv", wK_sb, 128), ("v", wV_sb, 64)):
                    ps = lgps.tile([128, 1024], F32, tag="lg", name=f"ps_{nm}_{h}")
                    for k in range(8):
                        # matmul out is capped at 512 fp32 (one PSUM bank):
                        # two column-halves (separate banks, each its own
                        # start/stop accumulation group), same weights
                        nc.tensor.matmul(ps[0:rows, 0:512], wsb[k][:, 0:rows],
                                         xfull[k][:, 1024 * h:1024 * h + 512],
                                         start=(k == 0), stop=(k == 7))
                        nc.tensor.matmul(ps[0:rows, 512:1024], wsb[k][:, 0:rows],
                                         xfull[k][:, 1024 * h + 512:1024 * h + 1024],
                                         start=(k == 0), stop=(k == 7))
                        yield
                    sb = wp.tile([rows, 1024], BF16, tag=f"sb_{nm}",
                                 name=f"sb_{nm}_{h}")
                    nc.vector.tensor_copy(sb[:], ps[0:rows, :])
                    outs[2 * h][nm] = sb[:, 0:512]
                    outs[2 * h + 1][nm] = sb[:, 512:1024]

            def gen_p1_pe(c, pr, aux):
                """stats matmuls, scale broadcasts, v transposes; Act ops inline."""
                qAs, qBs, ks, vs = pr["qA"], pr["qB"], pr["kv"], pr["v"]
                sqA = wp.tile([128, 512], BF16, tag="sq")
                nc.gpsimd.tensor_mul(sqA[:], qAs[:], qAs[:])
                sqB = wp.tile([128, 512], BF16, tag="sq")
                nc.gpsimd.tensor_mul(sqB[:], qBs[:], qBs[:])
                ms = mmps.tile([4, 512], F32, tag="mm", name=f"ms_{c}")
                nc.tensor.matmul(ms[:], e4b[:], sqA[:], start=True, stop=False)
                yield
                nc.tensor.matmul(ms[:], e4b[:], sqB[:], start=False, stop=True)
                yield
                msq = wp.tile([4, 512], F32, tag="msq")
                nc.scalar.activation(msq[:], ms[:], AF.Ln, bias=eps4[:], scale=1.0 / HD)
                wqr = wp.tile([4, 512], BF16, tag="wqr")
                nc.scalar.activation(wqr[:], msq[:], AF.Exp, bias=0.0, scale=-0.5)
                bc = mmps.tile([128, 512], F32, tag="mm", name=f"bc_{c}")
                nc.tensor.matmul(bc[:], sel32b[:], wqr[:], start=True, stop=True)
                yield
                bcs = wp.tile([128, 512], BF16, tag="bcs")
                nc.vector.tensor_copy(bcs[:], bc[:])
                aux["bcs"] = bcs
                sqk = wp.tile([64, 512], BF16, tag="sqk")
                nc.gpsimd.tensor_mul(sqk[:], ks[0:64, :], ks[0:64, :])
                msk = mmps.tile([1, 512], F32, tag="mm", name=f"msk_{c}")
                nc.tensor.matmul(msk[:], ones64_1[:], sqk[:], start=True, stop=True)
                yield
                msks = wp.tile([1, 512], F32, tag="msks")
                nc.scalar.activation(msks[:], msk[:], AF.Ln, bias=eps1[:], scale=1.0 / HD)
                u8row = wp.tile([1, 512], BF16, tag="u8row")
                nc.scalar.activation(u8row[:], msks[:], AF.Exp, bias=bln8_1[:], scale=-0.5)
                bcK = mmps.tile([64, 512], F32, tag="mm", name=f"bck_{c}")
                nc.tensor.matmul(bcK[:], ones1_64[:], u8row[:], start=True, stop=True)
                yield
                bcKs = wp.tile([64, 512], BF16, tag="bcks")
                nc.vector.tensor_copy(bcKs[:], bcK[:])
                aux["bcKs"] = bcKs
                for j in range(4):
                    tp = mmps.tile([128, 64], BF16, tag="mm", name=f"tp_{c}_{j}")
                    nc.tensor.transpose(tp[:], vs[0:64, 128 * j:128 * (j + 1)],
                                        identb[0:64, 0:64])
                    yield
                    nc.vector.tensor_copy(v_aug[4 * c + j][:, 0:64], tp[:])

            def gen_p1_k(c, pr, aux):
                """k rotary + scale on Pool; emitted before the q side so the
                next chunk's qk never head-of-line blocks on it."""
                sl = slice(512 * c, 512 * (c + 1))
                ks = pr["kv"]
                bcKs = aux["bcKs"]
                tk1 = wp.tile([64, 512], BF16, tag="tk")
                tk2 = wp.tile([64, 512], BF16, tag="tk")
                nc.gpsimd.tensor_mul(tk1[:], ks[0:64, :], cos_sb[0:64, sl])
                nc.gpsimd.tensor_mul(tk2[:], ks[64:128, :], sin_sb[64:128, sl])
                yield
                nc.gpsimd.tensor_add(tk1[:], tk1[:], tk2[:])
                nc.gpsimd.tensor_mul(kT2c[c][0:64, :], tk1[:], bcKs[:])
                yield
                # duplicate rows 64-127 via SBUF->SBUF DMA: partition
                # shifts are free on the DMA engine (gpsimd copy ~1.9us)
                nc.sync.dma_start(kT2c[c][64:128, :], kT2c[c][0:64, :])

            def gen_p1_q(c, pr, aux):
                """q rotary + qk-rmsnorm scale, both head pairs at once:
                full (128,512) DVE ops cost the same as the old (64,512)
                per-pair ones (the DVE is free-dim bound), so this is half
                the instructions.  Distinct tags per tq tile: tq1/tq2 are
                still read by the qpair muls after tq3 is allocated, so
                they must never share a pool slot ring."""
                sl = slice(512 * c, 512 * (c + 1))
                qAs, qBs = pr["qA"], pr["qB"]
                bcs = aux["bcs"]
                tq1 = wp.tile([128, 512], BF16, tag="tq1")
                tq2 = wp.tile([128, 512], BF16, tag="tq2")
                tq3 = wp.tile([128, 512], BF16, tag="tq3")
                nc.vector.tensor_mul(tq1[:], qAs[:], cos_sb[:, sl])
                nc.vector.tensor_mul(tq2[:], qBs[:], sin_sb[:, sl])
                yield
                nc.vector.tensor_add(tq1[:], tq1[:], tq2[:])
                nc.vector.tensor_mul(tq2[:], qBs[:], cos_sb[:, sl])
                yield
                nc.vector.tensor_mul(tq3[:], qAs[:], sin_sb[:, sl])
                nc.vector.tensor_sub(tq2[:], tq2[:], tq3[:])
                yield
                for hq in range(4):
                    r0 = 64 * (hq % 2)
                    nc.vector.tensor_mul(
                        qpairc[hq // 2][c][r0:r0 + 32, :],
                        tq1[32 * hq:32 * (hq + 1), :], bcs[32 * hq:32 * (hq + 1), :])
                    nc.vector.tensor_mul(
                        qpairc[hq // 2][c][r0 + 32:r0 + 64, :],
                        tq2[32 * hq:32 * (hq + 1), :], bcs[32 * hq:32 * (hq + 1), :])
                    yield

            def gen_wo(c4):
                for tt in range(4 * c4, 4 * c4 + 4):
                    for ch in range(2):
                        po = mmps.tile([128, 512], F32, tag="mm",
                                       name=f"po_{tt}_{ch}")
                        nc.tensor.matmul(po[:], yhat[0][:, 128 * tt:128 * (tt + 1)],
                                         Wo_sb[0][:, 512 * ch:512 * (ch + 1)],
                                         start=True, stop=False)
                        yield
                        nc.tensor.matmul(po[:], yhat[1][:, 128 * tt:128 * (tt + 1)],
                                         Wo_sb[1][:, 512 * ch:512 * (ch + 1)],
                                         start=False, stop=True)
                        yield
                        osb = osbp.tile([128, 512], BF16, tag="osb")
                        nc.vector.tensor_copy(osb[:], po[:])
                        nc.sync.dma_start(
                            outp[128 * tt:128 * (tt + 1), 512 * ch:512 * (ch + 1)],
                            osb[:])

            def emit_attn(c4, ppump=3.0):
                # fractional pump rate: accumulate credit so fillers spread
                # evenly across every (qk -> exp/pv) gap in this chunk
                credit = [0.0]

                def pump_r():
                    credit[0] += ppump
                    n = int(credit[0])
                    if n:
                        credit[0] -= n
                        pump(n)

                t0 = 512 * c4
                n_st = 4 * c4 + 4
                for pidx in range(2):
                    py = [pyps.tile([65, 512], F32, tag="py",
                                    name=f"py_{pidx}_{c4}_{hh}") for hh in range(2)]

                    def emit_qk(st):
                        qs = min(max(0, 128 * st - t0), 384)
                        lg = lgps.tile([128, 1024], F32, tag="lg",
                                       name=f"lg_{pidx}_{c4}_{st}")
                        for hh in range(2):
                            nc.tensor.matmul(
                                lg[:, 512 * hh + qs:512 * hh + 512],
                                kT2c[st // 4][64 * hh:64 * (hh + 1),
                                              128 * (st % 4):128 * (st % 4 + 1)],
                                qpairc[pidx][c4][64 * hh:64 * (hh + 1), qs:512],
                                start=True, stop=True,
                                tile_position=(64 * hh, 0))
                        return lg

                    def emit_exp(st, lg):
                        qs = min(max(0, 128 * st - t0), 384)
                        b0 = t0 + qs - 128 * st
                        wt = wtp.tile([128, 1024], BF16, tag="wt",
                                      name=f"wt_{pidx}_{c4}_{st}")
                        if qs == 0:
                            nc.scalar.activation(wt[:], lg[:], AF.Exp,
                                                 bias=0.0, scale=1.0)
                        else:
                            lg2 = lg[:].rearrange("p (h q) -> p h q", h=2)
                            wt2 = wt[:].rearrange("p (h q) -> p h q", h=2)
                            nc.scalar.activation(
                                wt2[:, :, qs:512], lg2[:, :, qs:512],
                                AF.Exp, bias=0.0, scale=1.0)
                        if b0 < 127:
                            # only the 128-col diagonal block needs masking
                            # (b0 == 0 there for every diagonal tile; beyond
                            # it keys <= queries always) -- 4x less gpsimd
                            wt2m = wt[:].rearrange("p (h q) -> p h q", h=2)
                            nc.gpsimd.affine_select(
                                out=wt2m[:, :, qs:qs + 128],
                                in_=wt2m[:, :, qs:qs + 128],
                                compare_op=ALU.is_ge, fill=0.0, base=b0,
                                pattern=[[0, 2], [1, 128]],
                                channel_multiplier=-1)
                        return wt

                    def emit_pv(st, wt):
                        qs = min(max(0, 128 * st - t0), 384)
                        for hh in range(2):
                            nc.tensor.matmul(
                                py[hh][:, qs:512], v_aug[st][:],
                                wt[:, 512 * hh + qs:512 * hh + 512],
                                start=(st == 0), stop=(st == n_st - 1))

                    # pv runs two tiles behind qk (wt pool is 3 deep), so
                    # the PE never blocks on the Act exp stream and vice
                    # versa -- kills the 1-tile-lookahead ping-pong.
                    lg_prev = None
                    pv_q = []
                    for st in range(n_st):
                        lg = emit_qk(st)
                        pump_r()
                        if lg_prev is not None:
                            pv_q.append((lg_prev[0], emit_exp(*lg_prev)))
                        if len(pv_q) >= 3:
                            emit_pv(*pv_q.pop(0))
                        lg_prev = (st, lg)
                    pump_r()
                    pv_q.append((lg_prev[0], emit_exp(*lg_prev)))
                    for s_w in pv_q:
                        emit_pv(*s_w)

                    # eagerly evacuate py (frees the PSUM accumulators before
                    # the next sweep's first pv) ...
                    ysall = wp.tile([128, 512], BF16, tag="ys",
                                    name=f"ys_{pidx}_{c4}")
                    ddb = [wp.tile([1, 512], BF16, tag=f"dd{hh}",
                                   name=f"ddb_{pidx}_{c4}_{hh}")
                           for hh in range(2)]
                    for hh in range(2):
                        nc.vector.tensor_copy(ysall[64 * hh:64 * hh + 64, :],
                                              py[hh][0:64, :])
                        nc.vector.tensor_copy(ddb[hh][:], py[hh][64:65, :])
                    # ... then run the normalize math as a pumped filler so
                    # its PE broadcast never head-of-line blocks the next
                    # sweep's qk stream
                    fillers.append(gen_norm(pidx, c4, ysall, ddb, t0))

            def gen_norm(pidx, c4, ysall, ddb, t0):
                # --- normalize: yhat = ysall / d ---
                # 1/d computed as exp(-ln d) on Act (vectorized over the
                # broadcast tile; serial 1-lane DVE reciprocal is ~3.2us).
                bc2 = mmps.tile([128, 512], F32, tag="mm", name=f"bc2_{pidx}_{c4}")
                for hh in range(2):
                    nc.tensor.matmul(bc2[64 * hh:64 * hh + 64, :], ones1_64[:],
                                     ddb[hh][:], start=True, stop=True)
                    yield
                lnd = wp.tile([128, 512], F32, tag="lnd")
                nc.scalar.activation(lnd[:], bc2[:], AF.Ln, bias=0.0, scale=1.0)
                bc2s = wp.tile([128, 512], BF16, tag="bc2s")
                nc.scalar.activation(bc2s[:], lnd[:], AF.Exp, bias=0.0, scale=-1.0)
                yield
                nc.vector.tensor_mul(yhat[pidx][:, t0:t0 + 512],
                                     ysall[:], bc2s[:])

            # ---- emission: dense proj phase, then Act-paced attention ----
            # Phase 1 keeps the PE matmul stream contiguous (HAM warm-up and
            # no DMA/stat head-of-line stalls inside it) by staggering each
            # chunk's small stats matmuls behind the next chunk's projection
            # block.  Phase 2 is paced by the Act exp stream; qk/pv plus
            # wo/norm/p1 fillers keep the PE above the HAM activity floor.
            def run(g):
                for _ in g:
                    pass

            pr = [{} for _ in range(NT)]
            aux = [{} for _ in range(NT)]
            emit_xdma(0)
            nc.sync.dma_start(wBbig[:], WqB[:, :])
            nc.sync.dma_start(wKbig[:], Wkv[:, :])
            nc.sync.dma_start(wVbig[:], Wv[:, :])
            emit_aux_dma()
            run(gen_proj2(0, pr))
            emit_xdma(1)
            run(gen_p1_pe(0, pr[0], aux[0]))

            def prep(c):
                run(gen_p1_k(c, pr[c], aux[c]))
                run(gen_p1_q(c, pr[c], aux[c]))

            prep(0)
            run(gen_proj2(1, pr))
            run(gen_p1_pe(1, pr[1], aux[1]))
            # chunk-1 prep ahead of the chunk-2/3 stats: qpair(1) is what
            # gates attention(1)'s start through the saturated DVE queue,
            # while the stats' consumers (prep(2)/(3), sweeps 2/3) run much
            # later -- and their gpsimd sq-muls still finish well before
            # those sweeps' mask affines
            prep(1)
            run(gen_p1_pe(2, pr[2], aux[2]))
            run(gen_p1_pe(3, pr[3], aux[3]))
            emit_attn(0, ppump=0.5)
            fillers.append(gen_wo(0))
            prep(2)
            emit_attn(1, ppump=1.0)
            fillers.append(gen_wo(1))
            prep(3)
            emit_attn(2, ppump=0.8)
            fillers.append(gen_wo(2))
            emit_attn(3, ppump=0.65)
            fillers.append(gen_wo(3))
            drain_all()
    return nc


_nc_cache = None


def _get_nc():
    global _nc_cache
    if _nc_cache is None:
        _nc_cache = build_nc()
    return _nc_cache


def _w_img(W):
    # (1024, M) -> SBUF image (128, 8*M): img[p, M*k + m] = W[128k + p, m]
    M = W.shape[1]
    return np.ascontiguousarray(
        W.reshape(8, 128, M).transpose(1, 0, 2).reshape(128, 8 * M))


def make_in_maps(x, cos, sin, Wq, Wk, Wv, Wo):
    bf = ml_dtypes.bfloat16
    cosT = np.ascontiguousarray(cos[0, :, 0, :].T)   # (32, T)
    sinT = np.ascontiguousarray(sin[0, :, 0, :].T)
    cosT4 = np.ascontiguousarray(np.tile(cosT, (4, 1))).astype(bf)  # (128, T)
    sinT4 = np.ascontiguousarray(np.tile(sinT, (4, 1))).astype(bf)
    in_maps = []
    for c in range(8):
        b, g = c // 4, c % 4
        heads = [4 * g + i for i in range(4)]
        permA = [64 * h + d for h in heads for d in range(HH)]
        permB = [64 * h + HH + d for h in heads for d in range(HH)]
        WqAh = _w_img(Wq[:, permA]).astype(bf)
        WqBh = _w_img(Wq[:, permB]).astype(bf)
        kA = Wk[:, 64 * g:64 * g + HH]
        kB = Wk[:, 64 * g + HH:64 * (g + 1)]
        Wkvh = _w_img(np.concatenate([kA, kB, kB, -kA], axis=1)).astype(bf)
        Wvh = _w_img(Wv[:, 64 * g:64 * (g + 1)]).astype(bf)
        Wo2h = np.ascontiguousarray(
            Wo[256 * g:256 * (g + 1), :].reshape(2, 128, C)).astype(bf)
        # x SBUF image: xTs[p, 2048j + t] = xT[128j + p, t]
        xTh = _w_img(x[b].T).astype(bf)
        in_maps.append({
            "xTs": xTh, "cosT4": cosT4, "sinT4": sinT4,
            "WqA": WqAh, "WqB": WqBh, "Wkv": Wkvh, "Wv": Wvh, "Wo2": Wo2h,
        })
    return in_maps


def run(inputs, trace=False, **kwargs):
    nc = _get_nc()
    in_maps = make_in_maps(**inputs)
    res = run_bass_kernel_spmd(nc, in_maps, core_ids=list(range(8)),
                               trace=trace, **kwargs)
    outs = [res.results[c]["outp"].astype(np.float32) for c in range(8)]
    full = np.stack([
        outs[0] + outs[1] + outs[2] + outs[3],
        outs[4] + outs[5] + outs[6] + outs[7],
    ]).astype(np.float32)
    return full, res


def kernel(**inputs):
    out, _ = run({k: np.asarray(v) for k, v in inputs.items()}, trace=False)
    return out

